# revision 1
# baseline (speedup 1.0000x reference)
"""Trainium2 Bass kernel for AdaptSelfAttention (Transformer-XL style relative
position attention).

Shapes (hardcoded): B=4, L=1024, H=512, NH=8, HD=64.
Sharding: 32 (batch, head) pairs -> 8 cores; core c handles batch c//2 and the
4-head group c%2 (hidden slice of 256 columns).

Math per (b, n):
  q = query @ Wq + bq   (per-head slice)          [L, 64]
  v = value @ Wv + bv                              [L, 64]
  k = key slice                                    [L, 64]
  rel = emb @ Wr + br                              [2L, 64]  (emb = sinusoid const)
  S[q_,k_] = (q+rrb).k  +  (q+rwb).rel[L+k_-q_]  +  k.rel[L+q_-k_]  + c2[k_]
       (c2[k_] = k.br ; the q-side br term is constant per row -> softmax-
        invariant, dropped)
  out = softmax_k(S with key-mask) @ v

Kernel computes S^T tiles (k on partitions, q free):
  - AC^T via matmul (contraction d=64)
  - E-term: E_[k,l] = k.rel[l] tiles -> DRAM -> skewed (diagonal-AP) DMA re-read
    gives E_sh^T directly (bf16), DVE-added pre-exp
  - BD-term: B_[q,l] = (q+rwb).rel[l] tiles -> DRAM -> skew read (cast to f32)
    gives BD_sh (S-orientation) -> PE transpose-accumulate into the f32 S PSUM
  - c2 + key-mask enter as the per-partition bias of the exp() activation
  - AV: lhsT = [v | 1] so the softmax denominator is row 64 of the output PSUM
Pairs are software-pipelined: pair p+1's B_/E_ production is emitted between
pair p's skew reads and its ki-loop.
"""

import math
import sys

import numpy as np

sys.path.insert(0, "/opt/trn_rl_repo")

import concourse.bass as bass
import concourse.tile as tile
from concourse.tile_rust import add_dep_helper
from concourse import bacc, mybir
from concourse.bass_utils import run_bass_kernel_spmd

import ml_dtypes

BF16 = ml_dtypes.bfloat16

B, L, H, NH, HD = 4, 1024, 512, 8, 64
PITCH = 1152  # stored l-window width per row of the B_/E_ scratch
NEG = -1e30


def _get_embedding(max_len, dim):
    half = dim // 2
    freq = np.exp(np.arange(half, dtype=np.float64) * (-math.log(10000.0) / (half - 1)))
    pos = np.arange(-max_len, max_len, dtype=np.float64)
    ang = pos[:, None] * freq[None, :]
    return np.concatenate([np.sin(ang), np.cos(ang)], axis=1)


def build_body(tc, ins, outs):
    """Emit the per-core kernel. ins/outs: dicts of bass.AP over DRAM."""
    nc = tc.nc
    f32 = mybir.dt.float32
    bf16 = mybir.dt.bfloat16
    Ident = mybir.ActivationFunctionType.Identity
    Exp = mybir.ActivationFunctionType.Exp

    from contextlib import ExitStack

    ctx = ExitStack()
    with ctx:
        # ---- pools ----
        io = ctx.enter_context(tc.tile_pool(name="io", bufs=1))
        persist = ctx.enter_context(tc.tile_pool(name="persist", bufs=1))
        bdp = ctx.enter_context(tc.tile_pool(name="bdp", bufs=3))    # bf16 batches
        ep = ctx.enter_context(tc.tile_pool(name="ep", bufs=3))      # esh batches
        pp = ctx.enter_context(tc.tile_pool(name="pp", bufs=6))      # exp outputs
        stg = ctx.enter_context(tc.tile_pool(name="stg", bufs=8))
        sml = ctx.enter_context(tc.tile_pool(name="sml", bufs=4))
        # PSUM: stagePS 2x[128,512]=2; psS f32 2; psB bf16 2x1=2; psO 2 -> 8
        stagePS = ctx.enter_context(tc.tile_pool(name="stagePS", bufs=2, space="PSUM"))
        psS = ctx.enter_context(tc.tile_pool(name="psS", bufs=1, space="PSUM"))
        psB = ctx.enter_context(tc.tile_pool(name="psB", bufs=2, space="PSUM"))
        psO = ctx.enter_context(tc.tile_pool(name="psO", bufs=1, space="PSUM"))
        dscr = ctx.enter_context(tc.tile_pool(name="dscr", bufs=2, space="DRAM"))
        descr = ctx.enter_context(tc.tile_pool(name="descr", bufs=2, space="DRAM"))

        # ---- stage constant/weight inputs into SBUF ----
        ident = persist.tile([128, 128], bf16, tag="ident")
        nc.scalar.dma_start(ident[:], ins["ident_bf"])
        identf = persist.tile([128, 128], f32, tag="identf")
        nc.scalar.dma_start(identf[:], ins["ident_f32"])

        relT = persist.tile([128, 2056], bf16, tag="relT")
        nc.scalar.dma_start(relT[:, 0:2049], ins["relTa"])

        # Wq/Wv [512, 256] -> [128, 4*256]
        wq_sb = persist.tile([128, 1024], bf16, tag="wq")
        wv_sb = persist.tile([128, 1024], bf16, tag="wv")
        for k in range(4):
            nc.scalar.dma_start(wq_sb[:, k * 256:(k + 1) * 256],
                                ins["Wq"][k * 128:(k + 1) * 128, :])
            nc.scalar.dma_start(wv_sb[:, k * 256:(k + 1) * 256],
                                ins["Wv"][k * 128:(k + 1) * 128, :])
        # qT/vT [512, 1024] -> 4 tiles each
        qT_sb, vT_sb = [], []
        for k in range(4):
            t = io.tile([128, 1024], bf16, tag=f"qT{k}", name=f"qTs{k}")
            nc.scalar.dma_start(t[:], ins["qT"][k * 128:(k + 1) * 128, :])
            qT_sb.append(t)
        for k in range(4):
            t = io.tile([128, 1024], bf16, tag=f"vT{k}", name=f"vTs{k}")
            nc.scalar.dma_start(t[:], ins["vT"][k * 128:(k + 1) * 128, :])
            vT_sb.append(t)
        # kT [256, 1024] -> 2 tiles
        kT_sb = []
        for t_ in range(2):
            t = persist.tile([128, 1024], bf16, tag=f"kT{t_}", name=f"kTs{t_}")
            nc.scalar.dma_start(t[:], ins["kT"][t_ * 128:(t_ + 1) * 128, :])
            kT_sb.append(t)
        # small vectors
        bq_sb = sml.tile([128, 2], f32, tag="bq")
        nc.scalar.dma_start(bq_sb[:], ins["bq2"])
        brr_sb = sml.tile([128, 2], f32, tag="brr")
        nc.scalar.dma_start(brr_sb[:], ins["brr2"])
        brw_sb = sml.tile([128, 2], f32, tag="brw")
        nc.scalar.dma_start(brw_sb[:], ins["brw2"])
        mb_sb = sml.tile([128, 8], f32, tag="mb")
        nc.scalar.dma_start(mb_sb[:], ins["maskbias"])
        bv_sb = sml.tile([128, 256], f32, tag="bv")
        nc.scalar.dma_start(bv_sb[:], ins["bv128"])

        biasA = sml.tile([128, 2], f32, tag="biasA")  # bq + r_r_bias
        nc.vector.tensor_add(biasA[:], bq_sb[:], brr_sb[:])
        biasB = sml.tile([128, 2], f32, tag="biasB")  # bq + r_w_bias
        nc.vector.tensor_add(biasB[:], bq_sb[:], brw_sb[:])

        # ---- q projection: qrrT/grwT [2 x (128, 1024)] (d on partitions) ----
        qrrT = [persist.tile([128, 1024], bf16, tag=f"qrrT{i}", name=f"qrrT{i}")
                for i in range(2)]
        grwT = [persist.tile([128, 1024], bf16, tag=f"grwT{i}", name=f"grwT{i}")
                for i in range(2)]
        for t_ in range(2):
            for nh in range(2):
                ps = stagePS.tile([128, 512], f32, tag="ps")
                for k in range(4):
                    nc.tensor.matmul(
                        ps[:],
                        wq_sb[:, k * 256 + t_ * 128: k * 256 + (t_ + 1) * 128],
                        qT_sb[k][:, nh * 512:(nh + 1) * 512],
                        start=(k == 0), stop=(k == 3),
                    )
                nc.scalar.activation(qrrT[t_][:, nh * 512:(nh + 1) * 512], ps[:],
                                     Ident, bias=biasA[:, t_:t_ + 1], scale=1.0)
                nc.scalar.activation(grwT[t_][:, nh * 512:(nh + 1) * 512], ps[:],
                                     Ident, bias=biasB[:, t_:t_ + 1], scale=1.0)

        # ---- v projection -> v_sb tiles [128, 4*65] ([v_head | 1]) ----
        v_sb = []
        for lt in range(8):
            ps = stagePS.tile([128, 512], f32, tag="ps")
            for k in range(4):
                nc.tensor.matmul(
                    ps[:, 0:256],
                    vT_sb[k][:, lt * 128:(lt + 1) * 128],
                    wv_sb[:, k * 256:(k + 1) * 256],
                    start=(k == 0), stop=(k == 3),
                )
            nc.vector.tensor_add(ps[:, 0:256], ps[:, 0:256], bv_sb[:])
            vt = persist.tile([128, 260], bf16, tag=f"vsb{lt}", name=f"vsb{lt}")
            src = ps[:, 0:256].rearrange("p (h d) -> p h d", d=64)
            dst = vt[:].rearrange("p (h e) -> p h e", e=65)[:, :, 0:64]
            nc.vector.tensor_copy(dst, src)
            nc.vector.memset(vt[:].rearrange("p (h e) -> p h e", e=65)[:, :, 64:65], 1.0)
            v_sb.append(vt)

        copy_engines = [
            lambda o_, i_: nc.scalar.copy(o_, i_),
            lambda o_, i_: nc.vector.tensor_copy(o_, i_),
        ]

        # ---- per-(b,head) pair loop, software-pipelined ----
        pair_state = {}

        def produce_init(p):
            scrB = dscr.tile([1024, PITCH], bf16, tag="scrB", name=f"scrB{p}")
            scrE = descr.tile([1024, PITCH], bf16, tag="scrE", name=f"scrE{p}")
            c2 = sml.tile([128, 8], f32, tag="c2", name=f"c2_{p}")
            ebias = sml.tile([128, 8], f32, tag="ebias", name=f"ebias{p}")
            pair_state[p] = (scrB, scrE, c2, ebias)

        def produce_tile(p, src_sel, qi):
            """One B_ (src_sel=0) or E_ (1) tile of pair p -> DRAM scratch."""
            t_ = p // 2
            o = (p % 2) * 64
            scrB, scrE, c2, ebias = pair_state[p]
            scr = scrB if src_sel == 0 else scrE
            W0 = 897 - 128 * qi
            if src_sel == 0:
                lhs = grwT[t_][o:o + 64, qi * 128:(qi + 1) * 128]
            else:
                lhs = kT_sb[t_][o:o + 64, qi * 128:(qi + 1) * 128]
            sb = stg.tile([128, PITCH], bf16, tag="stg")
            for ci, (c0, cw) in enumerate(((0, 512), (512, 512), (1024, 128))):
                ps = stagePS.tile([128, 512], f32, tag="ps")
                nc.tensor.matmul(ps[:, 0:cw], lhs,
                                 relT[o:o + 64, W0 + c0:W0 + c0 + cw],
                                 start=True, stop=True)
                if src_sel == 1 and ci == 2:
                    nc.tensor.matmul(ps[:, 128:129], lhs,
                                     relT[o:o + 64, 2048:2049],
                                     start=True, stop=True)
                    nc.scalar.activation(c2[:, qi:qi + 1], ps[:, 128:129],
                                         Ident, bias=0.0, scale=1.0)
                eng = copy_engines[0 if (qi * 3 + ci) % 3 == 0 else 1]
                eng(sb[:, c0:c0 + cw], ps[:, 0:cw])
            nc.sync.dma_start(scr[qi * 128:(qi + 1) * 128, :], sb[:])

        def produce_fini(p):
            scrB, scrE, c2, ebias = pair_state[p]
            nc.vector.tensor_add(ebias[:], c2[:], mb_sb[:])

        def skew_reads(p):
            """Batched diagonal re-reads for pair p."""
            scrB, scrE, c2, ebias = pair_state[p]
            scrB_ap, scrE_ap = scrB[:], scrE[:]
            # BD (bf16, qi-major): [128, 8*1024]
            bdall = bdp.tile([128, 8192], bf16, tag="bd", name=f"bdall{p}")
            srcB = bass.AP(scrB_ap.tensor, scrB_ap.offset + 127,
                           [[PITCH - 1, 128], [128 * PITCH, 8], [1, 1024]])
            nc.sync.dma_start(bdall[:].rearrange("p (a b) -> p a b", a=8), srcB)
            # E (bf16, ki-major by construction)
            eshall = ep.tile([128, 8192], bf16, tag="esh", name=f"esh{p}")
            srcE = bass.AP(scrE_ap.tensor, scrE_ap.offset + 127,
                           [[PITCH - 1, 128], [128 * PITCH, 8], [1, 1024]])
            nc.sync.dma_start(eshall[:].rearrange("p (a b) -> p a b", a=8), srcE)
            return bdall, eshall

        def ki_iter(p, ki, bdall, eshall, outT):
            t_ = p // 2
            o = (p % 2) * 64
            kTs = kT_sb[t_]
            qrr = qrrT[t_]
            ebias = pair_state[p][3]
            BDT = psB.tile([128, 1024], bf16, tag="bdt")
            for qi in range(8):
                nc.tensor.matmul(
                    BDT[:, qi * 128:(qi + 1) * 128],
                    bdall[:, qi * 1024 + ki * 128: qi * 1024 + (ki + 1) * 128],
                    ident[:],
                    is_transpose=True, start=True, stop=True,
                )
            X = pp.tile([128, 1024], bf16, tag="y")
            nc.vector.tensor_add(X[:], BDT[:],
                                 eshall[:, ki * 1024:(ki + 1) * 1024])
            ST = psS.tile([128, 1024], f32, tag="s")
            for nh in range(2):
                nc.tensor.matmul(
                    ST[:, nh * 512:(nh + 1) * 512],
                    kTs[o:o + 64, ki * 128:(ki + 1) * 128],
                    qrr[o:o + 64, nh * 512:(nh + 1) * 512],
                    start=True, stop=True,
                )
            nc.vector.tensor_add(ST[:], ST[:], X[:])
            P = pp.tile([128, 1024], bf16, tag="p")
            nc.scalar.activation(P[:], ST[:], Exp,
                                 bias=ebias[:, ki:ki + 1], scale=1.0)
            for nh in range(2):
                nc.tensor.matmul(
                    outT[0:65, nh * 512:(nh + 1) * 512],
                    v_sb[ki][:, p * 65:p * 65 + 65],
                    P[:, nh * 512:(nh + 1) * 512],
                    start=(ki == 0), stop=(ki == 7),
                )

        def finalize(p, outT):
            oT = sml.tile([65, 1024], f32, tag="oT", name=f"oT{p}")
            nc.scalar.activation(oT[:], outT[:], Ident, bias=0.0, scale=1.0)
            for qi in range(8):
                psF = stagePS.tile([128, 512], f32, tag="ps")
                nc.tensor.matmul(psF[0:128, 0:65],
                                 oT[0:65, qi * 128:(qi + 1) * 128],
                                 identf[0:65, 0:65],
                                 is_transpose=True, start=True, stop=True)
                rec = sml.tile([128, 1], f32, tag="rec")
                nc.vector.reciprocal(rec[:], psF[:, 64:65])
                fin = sml.tile([128, 64], f32, tag="fin")
                nc.vector.tensor_scalar_mul(fin[:], psF[:, 0:64], rec[:, 0:1])
                nc.sync.dma_start(outs["out"][p, qi * 128:(qi + 1) * 128, :], fin[:])

        def produce(p):
            produce_init(p)
            for src_sel in range(2):
                for qi in range(8):
                    produce_tile(p, src_sel, qi)
            produce_fini(p)

        produce(0)
        reads = {0: skew_reads(0)}
        for p in range(4):
            bdall, eshall = reads.pop(p)
            if p < 3:
                produce(p + 1)
                reads[p + 1] = skew_reads(p + 1)
            outT = psO.tile([65, 1024], f32, tag="o", name=f"outT{p}")
            for ki in range(8):
                ki_iter(p, ki, bdall, eshall, outT)
            finalize(p, outT)


_CACHE = {}


def _build_nc():
    if "nc" in _CACHE:
        return _CACHE["nc"]
    nc = bacc.Bacc("TRN2", target_bir_lowering=False, debug=False,
                   enable_asserts=False, num_devices=8)
    f32 = mybir.dt.float32
    bf16 = mybir.dt.bfloat16
    ins = {}

    def di(name, shape, dt):
        ins[name] = nc.dram_tensor(name, shape, dt, kind="ExternalInput").ap()

    di("qT", [512, 1024], bf16)
    di("vT", [512, 1024], bf16)
    di("kT", [256, 1024], bf16)
    di("Wq", [512, 256], bf16)
    di("Wv", [512, 256], bf16)
    di("relTa", [128, 2049], bf16)
    di("bq2", [128, 2], f32)
    di("brr2", [128, 2], f32)
    di("brw2", [128, 2], f32)
    di("maskbias", [128, 8], f32)
    di("bv128", [128, 256], f32)
    di("ident_bf", [128, 128], bf16)
    di("ident_f32", [128, 128], f32)
    outs = {"out": nc.dram_tensor("out", [4, 1024, 64], f32, kind="ExternalOutput").ap()}

    with tile.TileContext(nc) as tc:
        build_body(tc, ins, outs)
    nc.compile()
    _CACHE["nc"] = nc
    return nc


def make_in_maps(query, key, value, w_q_w, w_q_b, w_v_w, w_v_b, w_r_w, w_r_b,
                 r_r_bias, r_w_bias, seq_len):
    emb = _get_embedding(L, H)
    rel = (emb @ w_r_w.astype(np.float64) + w_r_b.astype(np.float64))  # [2L, 64]
    relTa = np.zeros((128, 2049), dtype=BF16)
    relTa[0:64, 0:2048] = rel.T.astype(BF16)
    relTa[0:64, 2048] = w_r_b.astype(BF16)
    relTa[64:128, :] = relTa[0:64, :]

    ident_bf = np.eye(128, dtype=BF16)
    ident_f32 = np.eye(128, dtype=np.float32)
    seq_len = int(seq_len)
    in_maps = []
    for c in range(8):
        b, hg = c // 2, c % 2
        hs = 256 * hg
        heads = slice(4 * hg, 4 * hg + 4)
        mb = np.where((np.arange(1024) < seq_len), 0.0, NEG).astype(np.float32)
        in_maps.append({
            "qT": np.ascontiguousarray(query[b].T).astype(BF16),
            "vT": np.ascontiguousarray(value[b].T).astype(BF16),
            "kT": np.ascontiguousarray(key[b][:, hs:hs + 256].T).astype(BF16),
            "Wq": np.ascontiguousarray(w_q_w[:, hs:hs + 256]).astype(BF16),
            "Wv": np.ascontiguousarray(w_v_w[:, hs:hs + 256]).astype(BF16),
            "relTa": relTa,
            "bq2": np.ascontiguousarray(w_q_b[hs:hs + 256].reshape(2, 128).T).astype(np.float32),
            "brr2": np.ascontiguousarray(r_r_bias[heads].reshape(2, 128).T).astype(np.float32),
            "brw2": np.ascontiguousarray(r_w_bias[heads].reshape(2, 128).T).astype(np.float32),
            "maskbias": np.ascontiguousarray(mb.reshape(8, 128).T).astype(np.float32),
            "bv128": np.tile(w_v_b[hs:hs + 256][None, :], (128, 1)).astype(np.float32),
            "ident_bf": ident_bf,
            "ident_f32": ident_f32,
        })
    return in_maps


def kernel(query, key, value, w_q_w, w_q_b, w_v_w, w_v_b, w_r_w, w_r_b,
           r_r_bias, r_w_bias, seq_len, _trace=False):
    query = np.asarray(query); key = np.asarray(key); value = np.asarray(value)
    w_q_w = np.asarray(w_q_w); w_q_b = np.asarray(w_q_b)
    w_v_w = np.asarray(w_v_w); w_v_b = np.asarray(w_v_b)
    w_r_w = np.asarray(w_r_w); w_r_b = np.asarray(w_r_b)
    r_r_bias = np.asarray(r_r_bias); r_w_bias = np.asarray(r_w_bias)

    nc = _build_nc()
    in_maps = make_in_maps(query, key, value, w_q_w, w_q_b, w_v_w, w_v_b,
                           w_r_w, w_r_b, r_r_bias, r_w_bias, seq_len)
    res = run_bass_kernel_spmd(nc, in_maps, core_ids=list(range(8)), trace=_trace)
    out = np.zeros((B, L, H), dtype=np.float32)
    for c in range(8):
        b, hg = c // 2, c % 2
        o = res.results[c]["out"]  # [4, 1024, 64]
        for j in range(4):
            out[b][:, 256 * hg + 64 * j: 256 * hg + 64 * (j + 1)] = o[j]
    if _trace:
        return out, res
    return out



# revision 3
# speedup vs baseline: 1.7531x; 1.7531x over previous
"""Trainium2 Bass kernel for AdaptSelfAttention (Transformer-XL style relative
position attention) — v2.

Shapes (hardcoded): B=4, L=1024, H=512, NH=8, HD=64.
Sharding: 32 (batch, head) pairs -> 8 cores; core c handles batch c//2 and the
4-head group c%2 (hidden slice of 256 columns).

Math per (b, n):
  q = query @ Wq + bq   (per-head slice)          [L, 64]
  v = value @ Wv + bv                              [L, 64]
  k = key slice                                    [L, 64]
  rel = emb @ Wr + br                              [2L, 64]  (emb = sinusoid const)
  S[q_,k_] = (q+rrb).k  +  (q+rwb).rel[L+k_-q_]  +  k.rel[L+q_-k_]  + c2[k_]
       (c2[k_] = k.br ; the q-side br term is constant per row -> softmax-
        invariant, dropped)
  out = softmax_k(S with key-mask) @ v

v2 design:
  - B_/E_ scratch lives in SBUF (fp16); the shift is ONE SBUF->SBUF skew DMA
    per scratch (diagonal source AP) instead of a DRAM write + skew re-read.
  - S^T accumulates fully on PE: AC matmuls + BD transpose-matmuls (identity
    rhs) + E_sh injection (identity lhsT). No vector adds in the chain; the
    tail is exp (ACT) + AV (PE) only, one ki behind the head.
  - PSUM->SBUF produce copies are rotated over DVE/Pool/ACT.
  - produce(p+1) tiles interleave into kis 0-4 of pair p; skew(p+1) issues at
    ki=4 so its latency hides behind kis 5-7.
  - fp16 everywhere on the logit path (q/k/rel/scratch) for precision
    headroom; P stays bf16 (exp range), v stays bf16.
"""

import math
import sys

import numpy as np

sys.path.insert(0, "/opt/trn_rl_repo")

import concourse.bass as bass
import concourse.tile as tile
from concourse import bacc, mybir
from concourse.bass_utils import run_bass_kernel_spmd

import ml_dtypes

BF16 = ml_dtypes.bfloat16
F16 = np.float16

B, L, H, NH, HD = 4, 1024, 512, 8, 64
PITCH = 1152  # stored l-window width per row of the B_/E_ scratch
NEG = -1e30


def _get_embedding(max_len, dim):
    half = dim // 2
    freq = np.exp(np.arange(half, dtype=np.float64) * (-math.log(10000.0) / (half - 1)))
    pos = np.arange(-max_len, max_len, dtype=np.float64)
    ang = pos[:, None] * freq[None, :]
    return np.concatenate([np.sin(ang), np.cos(ang)], axis=1)


def build_body(tc, ins, outs):
    """Emit the per-core kernel. ins/outs: dicts of bass.AP over DRAM."""
    nc = tc.nc
    f32 = mybir.dt.float32
    bf16 = mybir.dt.bfloat16
    f16 = mybir.dt.float16
    Ident = mybir.ActivationFunctionType.Identity
    Exp = mybir.ActivationFunctionType.Exp

    from contextlib import ExitStack

    ctx = ExitStack()
    with ctx:
        # ---- pools ----
        persist = ctx.enter_context(tc.tile_pool(name="persist", bufs=1))
        sml = ctx.enter_context(tc.tile_pool(name="sml", bufs=2))
        # PSUM banks: stagePS 3x[128,512]=3; psS 3x[128,512]=3; psO 2 -> 8
        stagePS = ctx.enter_context(tc.tile_pool(name="stagePS", bufs=3, space="PSUM"))
        psS = ctx.enter_context(tc.tile_pool(name="psS", bufs=3, space="PSUM"))
        psO = ctx.enter_context(tc.tile_pool(name="psO", bufs=1, space="PSUM"))

        # ---- stage constant/weight inputs into SBUF (one DMA each) ----
        # relT/kT first: they gate the E-produce(0) prologue
        relT = persist.tile([128, 2056], f16, tag="relT")
        nc.scalar.dma_start(relT[:, 0:2049], ins["relTa"])

        kT_t = persist.tile([128, 2048], f16, tag="kT")
        nc.scalar.dma_start(kT_t[:], ins["kT"])
        kT_sb = [kT_t[:, 0:1024], kT_t[:, 1024:2048]]

        # Wq|Wv packed [128, 2048] (wq 4x256, wv 4x256)
        w_sb = persist.tile([128, 2048], f16, tag="w")
        nc.scalar.dma_start(w_sb[:], ins["Wqv"])
        wq_sb = w_sb[:, 0:1024]
        wv_sb = w_sb[:, 1024:2048]

        ident = persist.tile([128, 128], f16, tag="ident")
        nc.scalar.dma_start(ident[:], ins["ident_f16"])
        identf = persist.tile([128, 128], f32, tag="identf")
        nc.scalar.dma_start(identf[:], ins["ident_f32"])

        # smalls packed f32 [128, 270]: bq2|brr2|brw2|maskbias(8)|bv(256)
        smalls = sml.tile([128, 270], f32, tag="smalls")
        nc.scalar.dma_start(smalls[:], ins["smalls"])
        bq_sb = smalls[:, 0:2]
        brr_sb = smalls[:, 2:4]
        brw_sb = smalls[:, 4:6]
        mb_sb = smalls[:, 6:14]
        bv_sb = smalls[:, 14:270]

        biasA = sml.tile([128, 2], f32, tag="biasA")  # bq + r_r_bias
        nc.vector.tensor_add(biasA[:], bq_sb, brr_sb)
        biasB = sml.tile([128, 2], f32, tag="biasB")  # bq + r_w_bias
        nc.vector.tensor_add(biasB[:], bq_sb, brw_sb)

        qrrT = [persist.tile([128, 1024], f16, tag=f"qrrT{i}", name=f"qrrT{i}")
                for i in range(2)]
        grwT = [persist.tile([128, 1024], f16, tag=f"grwT{i}", name=f"grwT{i}")
                for i in range(2)]
        v_sb = []

        scrp = ctx.enter_context(tc.tile_pool(name="scrp", bufs=1))   # SBUF scratch
        bdp = ctx.enter_context(tc.tile_pool(name="bdp", bufs=2))     # skewed bd
        ep = ctx.enter_context(tc.tile_pool(name="ep", bufs=2))       # skewed esh
        pp = ctx.enter_context(tc.tile_pool(name="pp", bufs=4))       # exp outputs

        # produce-copy engines. GPSIMD/Pool cannot read PSUM on TRN2, so
        # the PSUM->SBUF copies rotate over DVE and ACT only.
        dve_copy = lambda o_, i_: nc.vector.tensor_copy(o_, i_)
        act_copy = lambda o_, i_: nc.scalar.copy(o_, i_)
        copy_rot = [[dve_copy, act_copy]]
        copy_ctr = [0]

        # ---- per-(b,head) pair machinery ----
        pair_state = {}

        def produce_init(p):
            # one scratch tile per (src_sel, qi) chunk so each chunk's skew
            # DMA read only depends on that chunk's copies
            scr = {(s, qi): scrp.tile([128, PITCH], f16, tag=f"scr{s}_{qi}",
                                      name=f"scr{p}_{s}_{qi}")
                   for s in range(2) for qi in range(8)}
            c2 = sml.tile([128, 8], f32, tag="c2", name=f"c2_{p}")
            ebias = sml.tile([128, 8], f32, tag="ebias", name=f"ebias{p}")
            bdall = bdp.tile([128, 8192], f16, tag="bd", name=f"bdall{p}")
            eshall = ep.tile([128, 8192], f16, tag="esh", name=f"esh{p}")
            pair_state[p] = (scr, c2, ebias, bdall, eshall)
            return bdall, eshall

        def produce_tile(p, src_sel, qi):
            """One B_ (src_sel=0) or E_ (1) tile of pair p -> SBUF scratch."""
            t_ = p // 2
            o = (p % 2) * 64
            scrd, c2, ebias, bdall, eshall = pair_state[p]
            scr = scrd[(src_sel, qi)]
            W0 = 897 - 128 * qi
            if src_sel == 0:
                lhs = grwT[t_][o:o + 64, qi * 128:(qi + 1) * 128]
            else:
                lhs = kT_sb[t_][o:o + 64, qi * 128:(qi + 1) * 128]
            for ci, (c0, cw) in enumerate(((0, 512), (512, 512), (1024, 128))):
                ps = stagePS.tile([128, 512], f32, tag="ps")
                nc.tensor.matmul(ps[:, 0:cw], lhs,
                                 relT[o:o + 64, W0 + c0:W0 + c0 + cw],
                                 start=True, stop=True)
                if src_sel == 1 and ci == 2:
                    nc.tensor.matmul(ps[:, 128:129], lhs,
                                     relT[o:o + 64, 2048:2049],
                                     start=True, stop=True)
                    nc.vector.tensor_copy(c2[:, qi:qi + 1], ps[:, 128:129])
                rot = copy_rot[0]
                eng = rot[copy_ctr[0] % len(rot)]
                copy_ctr[0] += 1
                eng(scr[:][:, c0:c0 + cw], ps[:, 0:cw])
            # this chunk of the scratch is complete: issue its skew DMA now
            # skew: dst[p', i] = scr[p', 127 - p' + i]
            dst = bdall if src_sel == 0 else eshall
            scr_ap = scr[:]
            src_ap = bass.AP(scr_ap.tensor, scr_ap.offset + 127,
                             [[PITCH - 1, 128], [1, 1024]])
            nc.sync.dma_start(dst[:, qi * 1024:(qi + 1) * 1024], src_ap)

        def produce_fini(p):
            c2, ebias = pair_state[p][1], pair_state[p][2]
            nc.vector.tensor_add(ebias[:], c2[:], mb_sb)

        def ki_half_head(p, ki, h, bdall, eshall):
            """One 512-col half of ST = AC^T + BD_sh^T + E_sh^T (all PE)."""
            t_ = p // 2
            o = (p % 2) * 64
            kTs = kT_sb[t_]
            qrr = qrrT[t_]
            STh = psS.tile([128, 512], f32, tag="sh", name=f"ST{p}_{ki}_{h}")
            nc.tensor.matmul(
                STh[:],
                kTs[o:o + 64, ki * 128:(ki + 1) * 128],
                qrr[o:o + 64, h * 512:(h + 1) * 512],
                start=True, stop=False,
            )
            for j in range(4):
                qi = 4 * h + j
                nc.tensor.matmul(
                    STh[:, j * 128:(j + 1) * 128],
                    bdall[:, qi * 1024 + ki * 128: qi * 1024 + (ki + 1) * 128],
                    ident[:],
                    start=False, stop=False,
                )
            nc.tensor.matmul(
                STh[:],
                ident[:],
                eshall[:, ki * 1024 + h * 512: ki * 1024 + (h + 1) * 512],
                start=False, stop=True,
            )
            return STh

        def ki_half_tail(p, ki, h, STh, outT):
            ebias = pair_state[p][2]
            Ph = pp.tile([128, 512], bf16, tag="ph", name=f"P{p}_{ki}_{h}")
            nc.scalar.activation(Ph[:], STh[:], Exp,
                                 bias=ebias[:, ki:ki + 1], scale=1.0)
            nc.tensor.matmul(
                outT[0:65, h * 512:(h + 1) * 512],
                v_sb[ki][:, p * 65:p * 65 + 65],
                Ph[:],
                start=(ki == 0), stop=(ki == 7),
            )

        def finalize(p, outT):
            oT = sml.tile([65, 1024], f32, tag="oT", name=f"oT{p}")
            nc.scalar.activation(oT[:], outT[:], Ident, bias=0.0, scale=1.0)
            fin = sml.tile([128, 512], f32, tag="fin", name=f"fin{p}")
            for qi in range(8):
                psF = stagePS.tile([128, 512], f32, tag="ps")
                nc.tensor.matmul(psF[0:128, 0:65],
                                 oT[0:65, qi * 128:(qi + 1) * 128],
                                 identf[0:65, 0:65],
                                 is_transpose=True, start=True, stop=True)
                rec = sml.tile([128, 1], f32, tag="rec")
                nc.vector.reciprocal(rec[:], psF[:, 64:65])
                nc.vector.tensor_scalar_mul(fin[:, qi * 64:(qi + 1) * 64],
                                            psF[:, 0:64], rec[:, 0:1])
            nc.sync.dma_start(outs["out"][p],
                              fin[:].rearrange("p (a d) -> p a d", a=8))

        # ================= emission schedule =================
        # E-produce(0) first (needs only kT/relT); q-proj groups interleave
        # once qT arrives; B(0) and v-proj interleave after.
        produce_init(0)

        io_ctx = ExitStack()
        io = io_ctx.enter_context(tc.tile_pool(name="io", bufs=1))
        qT_sb = io.tile([128, 4096], f16, tag="qT")
        nc.scalar.dma_start(qT_sb[:, 0:2048], ins["qT"][:, 0:2048])
        nc.scalar.dma_start(qT_sb[:, 2048:4096], ins["qT"][:, 2048:4096])
        qT_halves = [qT_sb[:, 0:2048], qT_sb[:, 2048:4096]]
        vT_sb = io.tile([128, 4096], f16, tag="vT")
        nc.scalar.dma_start(vT_sb[:, 0:2048], ins["vT"][:, 0:2048])
        nc.scalar.dma_start(vT_sb[:, 2048:4096], ins["vT"][:, 2048:4096])
        vT_halves = [vT_sb[:, 0:2048], vT_sb[:, 2048:4096]]

        def q_proj_group(t_, nh):
            ps = stagePS.tile([128, 512], f32, tag="ps")
            for k in range(4):
                qh = qT_halves[k // 2]
                c0 = (k % 2) * 1024 + nh * 512
                nc.tensor.matmul(
                    ps[:],
                    wq_sb[:, k * 256 + t_ * 128: k * 256 + (t_ + 1) * 128],
                    qh[:, c0: c0 + 512],
                    start=(k == 0), stop=(k == 3),
                )
            nc.scalar.activation(qrrT[t_][:, nh * 512:(nh + 1) * 512], ps[:],
                                 Ident, bias=biasA[:, t_:t_ + 1], scale=1.0)
            nc.vector.tensor_scalar_add(grwT[t_][:, nh * 512:(nh + 1) * 512],
                                        ps[:], biasB[:, t_:t_ + 1])

        def v_proj_group(lt):
            ps = stagePS.tile([128, 512], f32, tag="ps")
            for k in range(4):
                vh = vT_halves[k // 2]
                c0 = (k % 2) * 1024 + lt * 128
                nc.tensor.matmul(
                    ps[:, 0:256],
                    vh[:, c0: c0 + 128],
                    wv_sb[:, k * 256:(k + 1) * 256],
                    start=(k == 0), stop=(k == 3),
                )
            nc.vector.tensor_add(ps[:, 0:256], ps[:, 0:256], bv_sb)
            vt = persist.tile([128, 260], bf16, tag=f"vsb{lt}", name=f"vsb{lt}")
            vsrc = ps[:, 0:256].rearrange("p (h d) -> p h d", d=64)
            vdst = vt[:].rearrange("p (h e) -> p h e", e=65)[:, :, 0:64]
            nc.vector.tensor_copy(vdst, vsrc)
            nc.vector.memset(vt[:].rearrange("p (h e) -> p h e", e=65)[:, :, 64:65], 1.0)
            v_sb.append(vt)

        for qi in range(4):
            produce_tile(0, 1, qi)
        q_proj_group(0, 0)
        produce_tile(0, 1, 4)
        q_proj_group(0, 1)
        produce_tile(0, 1, 5)
        q_proj_group(1, 0)
        produce_tile(0, 1, 6)
        q_proj_group(1, 1)
        produce_tile(0, 1, 7)

        # B-produce(0) interleaved with v-proj
        copy_rot[0] = [dve_copy, act_copy, dve_copy]
        for qi in range(8):
            produce_tile(0, 0, qi)
            v_proj_group(qi)
        io_ctx.close()

        produce_fini(0)

        # ---- pair loop: pair p emits pair p+1's produce tiles ----
        for p in range(4):
            bdall, eshall = pair_state[p][3], pair_state[p][4]
            outT = psO.tile([65, 1024], f32, tag="o", name=f"outT{p}")
            prodlist = []
            if p < 3:
                produce_init(p + 1)
                prodlist = ([(p + 1, 1, qi) for qi in range(8)] +
                            [(p + 1, 0, qi) for qi in range(8)])
            STs = {}
            pi = 0
            for ki in range(8):
                for h in range(2):
                    STs[(ki, h)] = ki_half_head(p, ki, h, bdall, eshall)
                    if ki > 0:
                        ki_half_tail(p, ki - 1, h, STs.pop((ki - 1, h)), outT)
                    if pi < len(prodlist):
                        tp, s, qi = prodlist[pi]; pi += 1
                        produce_tile(tp, s, qi)
                if p < 3 and ki == 7:
                    produce_fini(p + 1)
            for h in range(2):
                ki_half_tail(p, 7, h, STs.pop((7, h)), outT)
            finalize(p, outT)


_CACHE = {}


def _build_nc():
    if "nc" in _CACHE:
        return _CACHE["nc"]
    nc = bacc.Bacc("TRN2", target_bir_lowering=False, debug=False,
                   enable_asserts=False, num_devices=8)
    f32 = mybir.dt.float32
    f16 = mybir.dt.float16
    ins = {}

    def di(name, shape, dt):
        ins[name] = nc.dram_tensor(name, shape, dt, kind="ExternalInput").ap()

    di("qT", [128, 4096], f16)
    di("vT", [128, 4096], f16)
    di("kT", [128, 2048], f16)
    di("Wqv", [128, 2048], f16)
    di("relTa", [128, 2049], f16)
    di("smalls", [128, 270], f32)
    di("ident_f16", [128, 128], f16)
    di("ident_f32", [128, 128], f32)
    outs = {"out": nc.dram_tensor("out", [4, 128, 8, 64], f32,
                                  kind="ExternalOutput").ap()}

    with tile.TileContext(nc) as tc:
        build_body(tc, ins, outs)
    nc.compile()
    _CACHE["nc"] = nc
    return nc


def make_in_maps(query, key, value, w_q_w, w_q_b, w_v_w, w_v_b, w_r_w, w_r_b,
                 r_r_bias, r_w_bias, seq_len):
    emb = _get_embedding(L, H)
    rel = (emb @ w_r_w.astype(np.float64) + w_r_b.astype(np.float64))  # [2L, 64]
    relTa = np.zeros((128, 2049), dtype=F16)
    relTa[0:64, 0:2048] = rel.T.astype(F16)
    relTa[0:64, 2048] = w_r_b.astype(F16)
    relTa[64:128, :] = relTa[0:64, :]

    ident_f16 = np.eye(128, dtype=F16)
    ident_f32 = np.eye(128, dtype=np.float32)
    seq_len = int(seq_len)
    in_maps = []
    for c in range(8):
        b, hg = c // 2, c % 2
        hs = 256 * hg
        heads = slice(4 * hg, 4 * hg + 4)
        mb = np.where((np.arange(1024) < seq_len), 0.0, NEG).astype(np.float32)
        qT = np.ascontiguousarray(query[b].T).astype(F16)       # [512, 1024]
        vT = np.ascontiguousarray(value[b].T).astype(F16)
        kT = np.ascontiguousarray(key[b][:, hs:hs + 256].T).astype(F16)
        Wq = np.ascontiguousarray(w_q_w[:, hs:hs + 256]).astype(F16)  # [512,256]
        Wv = np.ascontiguousarray(w_v_w[:, hs:hs + 256]).astype(F16)
        # pack: qT -> [128, 4096] (4 chunks of 128 rows side by side)
        qTp = qT.reshape(4, 128, 1024).transpose(1, 0, 2).reshape(128, 4096)
        vTp = vT.reshape(4, 128, 1024).transpose(1, 0, 2).reshape(128, 4096)
        kTp = kT.reshape(2, 128, 1024).transpose(1, 0, 2).reshape(128, 2048)
        Wqp = Wq.reshape(4, 128, 256).transpose(1, 0, 2).reshape(128, 1024)
        Wvp = Wv.reshape(4, 128, 256).transpose(1, 0, 2).reshape(128, 1024)
        smalls = np.zeros((128, 270), dtype=np.float32)
        smalls[:, 0:2] = w_q_b[hs:hs + 256].reshape(2, 128).T
        smalls[:, 2:4] = r_r_bias[heads].reshape(2, 128).T
        smalls[:, 4:6] = r_w_bias[heads].reshape(2, 128).T
        smalls[:, 6:14] = mb.reshape(8, 128).T
        smalls[:, 14:270] = np.tile(w_v_b[hs:hs + 256][None, :], (128, 1))
        in_maps.append({
            "qT": qTp,
            "vT": vTp,
            "kT": kTp,
            "Wqv": np.concatenate([Wqp, Wvp], axis=1),
            "relTa": relTa,
            "smalls": smalls,
            "ident_f16": ident_f16,
            "ident_f32": ident_f32,
        })
    return in_maps


def kernel(query, key, value, w_q_w, w_q_b, w_v_w, w_v_b, w_r_w, w_r_b,
           r_r_bias, r_w_bias, seq_len, _trace=False):
    query = np.asarray(query); key = np.asarray(key); value = np.asarray(value)
    w_q_w = np.asarray(w_q_w); w_q_b = np.asarray(w_q_b)
    w_v_w = np.asarray(w_v_w); w_v_b = np.asarray(w_v_b)
    w_r_w = np.asarray(w_r_w); w_r_b = np.asarray(w_r_b)
    r_r_bias = np.asarray(r_r_bias); r_w_bias = np.asarray(r_w_bias)

    nc = _build_nc()
    in_maps = make_in_maps(query, key, value, w_q_w, w_q_b, w_v_w, w_v_b,
                           w_r_w, w_r_b, r_r_bias, r_w_bias, seq_len)
    res = run_bass_kernel_spmd(nc, in_maps, core_ids=list(range(8)), trace=_trace)
    out = np.zeros((B, L, H), dtype=np.float32)
    for c in range(8):
        b, hg = c // 2, c % 2
        o = res.results[c]["out"]  # [4, 128, 8, 64] (pair, q-in-tile, qi, d)
        for j in range(4):
            out[b][:, 256 * hg + 64 * j: 256 * hg + 64 * (j + 1)] = (
                o[j].transpose(1, 0, 2).reshape(1024, 64))
    if _trace:
        return out, res
    return out


# revision 4
# speedup vs baseline: 1.8588x; 1.0603x over previous
"""Trainium2 Bass kernel for AdaptSelfAttention (Transformer-XL style relative
position attention) — v2.

Shapes (hardcoded): B=4, L=1024, H=512, NH=8, HD=64.
Sharding: 32 (batch, head) pairs -> 8 cores; core c handles batch c//2 and the
4-head group c%2 (hidden slice of 256 columns).

Math per (b, n):
  q = query @ Wq + bq   (per-head slice)          [L, 64]
  v = value @ Wv + bv                              [L, 64]
  k = key slice                                    [L, 64]
  rel = emb @ Wr + br                              [2L, 64]  (emb = sinusoid const)
  S[q_,k_] = (q+rrb).k  +  (q+rwb).rel[L+k_-q_]  +  k.rel[L+q_-k_]  + c2[k_]
       (c2[k_] = k.br ; the q-side br term is constant per row -> softmax-
        invariant, dropped)
  out = softmax_k(S with key-mask) @ v

v2 design:
  - B_/E_ scratch lives in SBUF (fp16); the shift is ONE SBUF->SBUF skew DMA
    per scratch (diagonal source AP) instead of a DRAM write + skew re-read.
  - S^T accumulates fully on PE: AC matmuls + BD transpose-matmuls (identity
    rhs) + E_sh injection (identity lhsT). No vector adds in the chain; the
    tail is exp (ACT) + AV (PE) only, one ki behind the head.
  - PSUM->SBUF produce copies are rotated over DVE/Pool/ACT.
  - produce(p+1) tiles interleave into kis 0-4 of pair p; skew(p+1) issues at
    ki=4 so its latency hides behind kis 5-7.
  - fp16 everywhere on the logit path (q/k/rel/scratch) for precision
    headroom; P stays bf16 (exp range), v stays bf16.
"""

import math
import sys

import numpy as np

sys.path.insert(0, "/opt/trn_rl_repo")

import concourse.bass as bass
import concourse.tile as tile
from concourse import bacc, mybir
from concourse.bass_utils import run_bass_kernel_spmd

import ml_dtypes

BF16 = ml_dtypes.bfloat16
F16 = np.float16

B, L, H, NH, HD = 4, 1024, 512, 8, 64
PITCH = 1152  # stored l-window width per row of the B_/E_ scratch
NEG = -1e30


def _get_embedding(max_len, dim):
    half = dim // 2
    freq = np.exp(np.arange(half, dtype=np.float64) * (-math.log(10000.0) / (half - 1)))
    pos = np.arange(-max_len, max_len, dtype=np.float64)
    ang = pos[:, None] * freq[None, :]
    return np.concatenate([np.sin(ang), np.cos(ang)], axis=1)


def build_body(tc, ins, outs):
    """Emit the per-core kernel. ins/outs: dicts of bass.AP over DRAM."""
    nc = tc.nc
    f32 = mybir.dt.float32
    bf16 = mybir.dt.bfloat16
    f16 = mybir.dt.float16
    Ident = mybir.ActivationFunctionType.Identity
    Exp = mybir.ActivationFunctionType.Exp

    from contextlib import ExitStack

    ctx = ExitStack()
    with ctx:
        # ---- pools ----
        persist = ctx.enter_context(tc.tile_pool(name="persist", bufs=1))
        sml = ctx.enter_context(tc.tile_pool(name="sml", bufs=2))
        # PSUM banks: stagePS 3x[128,512]=3; psS 3x[128,512]=3; psO 2 -> 8
        stagePS = ctx.enter_context(tc.tile_pool(name="stagePS", bufs=3, space="PSUM"))
        psS = ctx.enter_context(tc.tile_pool(name="psS", bufs=3, space="PSUM"))
        psO = ctx.enter_context(tc.tile_pool(name="psO", bufs=1, space="PSUM"))

        # ---- stage constant/weight inputs into SBUF (one DMA each) ----
        # relT/kT first: they gate the E-produce(0) prologue
        relT = persist.tile([128, 2056], f16, tag="relT")
        nc.scalar.dma_start(relT[:, 0:2049], ins["relTa"])

        kT_t = persist.tile([128, 2048], f16, tag="kT")
        nc.scalar.dma_start(kT_t[:], ins["kT"])
        kT_sb = [kT_t[:, 0:1024], kT_t[:, 1024:2048]]

        # Wq|Wv packed [128, 2048] (wq 4x256, wv 4x256)
        w_sb = persist.tile([128, 2048], f16, tag="w")
        nc.scalar.dma_start(w_sb[:], ins["Wqv"])
        wq_sb = w_sb[:, 0:1024]
        wv_sb = w_sb[:, 1024:2048]

        ident = persist.tile([128, 128], f16, tag="ident")
        nc.scalar.dma_start(ident[:], ins["ident_f16"])
        identf = persist.tile([128, 128], f32, tag="identf")
        nc.scalar.dma_start(identf[:], ins["ident_f32"])

        # smalls packed f32 [128, 270]: bq2|brr2|brw2|maskbias(8)|bv(256)
        smalls = sml.tile([128, 270], f32, tag="smalls")
        nc.scalar.dma_start(smalls[:], ins["smalls"])
        bq_sb = smalls[:, 0:2]
        brr_sb = smalls[:, 2:4]
        brw_sb = smalls[:, 4:6]
        mb_sb = smalls[:, 6:14]
        bv_sb = smalls[:, 14:270]

        biasA = sml.tile([128, 2], f32, tag="biasA")  # bq + r_r_bias
        nc.vector.tensor_add(biasA[:], bq_sb, brr_sb)
        biasB = sml.tile([128, 2], f32, tag="biasB")  # bq + r_w_bias
        nc.vector.tensor_add(biasB[:], bq_sb, brw_sb)

        qrrT = [persist.tile([128, 1024], f16, tag=f"qrrT{i}", name=f"qrrT{i}")
                for i in range(2)]
        grwT = [persist.tile([128, 1024], f16, tag=f"grwT{i}", name=f"grwT{i}")
                for i in range(2)]
        v_sb = []

        scrp = ctx.enter_context(tc.tile_pool(name="scrp", bufs=1))   # SBUF scratch
        bdp = ctx.enter_context(tc.tile_pool(name="bdp", bufs=2))     # skewed bd
        ep = ctx.enter_context(tc.tile_pool(name="ep", bufs=2))       # skewed esh
        pp = ctx.enter_context(tc.tile_pool(name="pp", bufs=4))       # exp outputs

        # produce-copy engines. GPSIMD/Pool cannot read PSUM on TRN2, so
        # the PSUM->SBUF copies rotate over DVE and ACT only.
        dve_copy = lambda o_, i_: nc.vector.tensor_copy(o_, i_)
        act_copy = lambda o_, i_: nc.scalar.copy(o_, i_)
        copy_rot = [[dve_copy, act_copy]]
        copy_ctr = [0]

        # ---- per-(b,head) pair machinery ----
        pair_state = {}

        def produce_init(p):
            # one scratch tile per (src_sel, qi) chunk so each chunk's skew
            # DMA read only depends on that chunk's copies
            scr = {(s, qi): scrp.tile([128, PITCH], f16, tag=f"scr{s}_{qi}",
                                      name=f"scr{p}_{s}_{qi}")
                   for s in range(2) for qi in range(8)}
            c2 = sml.tile([128, 8], f32, tag="c2", name=f"c2_{p}")
            ebias = sml.tile([128, 8], f32, tag="ebias", name=f"ebias{p}")
            bdall = bdp.tile([128, 8192], f16, tag="bd", name=f"bdall{p}")
            eshall = ep.tile([128, 8192], f16, tag="esh", name=f"esh{p}")
            pair_state[p] = (scr, c2, ebias, bdall, eshall)
            return bdall, eshall

        def produce_tile(p, src_sel, qi):
            """One B_ (src_sel=0) or E_ (1) tile of pair p -> SBUF scratch."""
            t_ = p // 2
            o = (p % 2) * 64
            scrd, c2, ebias, bdall, eshall = pair_state[p]
            scr = scrd[(src_sel, qi)]
            W0 = 897 - 128 * qi
            if src_sel == 0:
                lhs = grwT[t_][o:o + 64, qi * 128:(qi + 1) * 128]
            else:
                lhs = kT_sb[t_][o:o + 64, qi * 128:(qi + 1) * 128]
            for ci, (c0, cw) in enumerate(((0, 512), (512, 512), (1024, 128))):
                ps = stagePS.tile([128, 512], f32, tag="ps")
                nc.tensor.matmul(ps[:, 0:cw], lhs,
                                 relT[o:o + 64, W0 + c0:W0 + c0 + cw],
                                 start=True, stop=True)
                if src_sel == 1 and ci == 2:
                    nc.tensor.matmul(ps[:, 128:129], lhs,
                                     relT[o:o + 64, 2048:2049],
                                     start=True, stop=True)
                    nc.vector.tensor_copy(c2[:, qi:qi + 1], ps[:, 128:129])
                rot = copy_rot[0]
                eng = rot[copy_ctr[0] % len(rot)]
                copy_ctr[0] += 1
                eng(scr[:][:, c0:c0 + cw], ps[:, 0:cw])
            # this chunk of the scratch is complete: issue its skew DMA now
            # skew: dst[p', i] = scr[p', 127 - p' + i]
            dst = bdall if src_sel == 0 else eshall
            scr_ap = scr[:]
            src_ap = bass.AP(scr_ap.tensor, scr_ap.offset + 127,
                             [[PITCH - 1, 128], [1, 1024]])
            nc.sync.dma_start(dst[:, qi * 1024:(qi + 1) * 1024], src_ap)

        def produce_fini(p):
            c2, ebias = pair_state[p][1], pair_state[p][2]
            nc.vector.tensor_add(ebias[:], c2[:], mb_sb)

        def ki_half_head(p, ki, h, bdall, eshall):
            """One 512-col half of ST = AC^T + BD_sh^T + E_sh^T (all PE)."""
            t_ = p // 2
            o = (p % 2) * 64
            kTs = kT_sb[t_]
            STh = psS.tile([128, 512], f32, tag="sh", name=f"ST{p}_{ki}_{h}")
            nc.tensor.matmul(
                STh[:],
                kTs[o:o + 64, ki * 128:(ki + 1) * 128],
                qrrT[t_][o:o + 64, h * 512:(h + 1) * 512],
                start=True, stop=False,
            )
            for j in range(4):
                qi = 4 * h + j
                nc.tensor.matmul(
                    STh[:, j * 128:(j + 1) * 128],
                    bdall[:, qi * 1024 + ki * 128: qi * 1024 + (ki + 1) * 128],
                    ident[:],
                    start=False, stop=False,
                )
            nc.tensor.matmul(
                STh[:],
                ident[:],
                eshall[:, ki * 1024 + h * 512: ki * 1024 + (h + 1) * 512],
                start=False, stop=True,
            )
            return STh

        def ki_half_tail(p, ki, h, STh, outT):
            ebias = pair_state[p][2]
            Ph = pp.tile([128, 512], bf16, tag="ph", name=f"P{p}_{ki}_{h}")
            nc.scalar.activation(Ph[:], STh[:], Exp,
                                 bias=ebias[:, ki:ki + 1], scale=1.0)
            nc.tensor.matmul(
                outT[0:65, h * 512:(h + 1) * 512],
                v_sb[ki][:, p * 65:p * 65 + 65],
                Ph[:],
                start=(ki == 0), stop=(ki == 7),
            )

        def finalize(p, outT):
            # ship the raw [65, 1024] accumulator ([attn@vWv ; denom] rows);
            # the host divides, transposes and adds bv
            oT = sml.tile([65, 1024], f32, tag="oT", name=f"oT{p}")
            nc.vector.tensor_copy(oT[:], outT[:])
            nc.sync.dma_start(outs["out"][p], oT[:])

        # ================= emission schedule =================
        # E-produce(0) first (needs only kT/relT); q-proj groups interleave
        # once qT arrives; B(0) and v-proj interleave after.
        produce_init(0)

        io_ctx = ExitStack()
        io = io_ctx.enter_context(tc.tile_pool(name="io", bufs=1))
        qT_sb = io.tile([128, 4096], f16, tag="qT")
        nc.scalar.dma_start(qT_sb[:, 0:2048], ins["qT"][:, 0:2048])
        nc.scalar.dma_start(qT_sb[:, 2048:4096], ins["qT"][:, 2048:4096])
        qT_halves = [qT_sb[:, 0:2048], qT_sb[:, 2048:4096]]
        vT_sb = io.tile([128, 4096], f16, tag="vT")
        nc.scalar.dma_start(vT_sb[:, 0:2048], ins["vT"][:, 0:2048])
        nc.scalar.dma_start(vT_sb[:, 2048:4096], ins["vT"][:, 2048:4096])
        vT_halves = [vT_sb[:, 0:2048], vT_sb[:, 2048:4096]]

        def q_proj_group(t_, nh):
            ps = stagePS.tile([128, 512], f32, tag="ps")
            for k in range(4):
                qh = qT_halves[k // 2]
                c0 = (k % 2) * 1024 + nh * 512
                nc.tensor.matmul(
                    ps[:],
                    wq_sb[:, k * 256 + t_ * 128: k * 256 + (t_ + 1) * 128],
                    qh[:, c0: c0 + 512],
                    start=(k == 0), stop=(k == 3),
                )
            nc.scalar.activation(qrrT[t_][:, nh * 512:(nh + 1) * 512], ps[:],
                                 Ident, bias=biasA[:, t_:t_ + 1], scale=1.0)
            nc.scalar.activation(grwT[t_][:, nh * 512:(nh + 1) * 512], ps[:],
                                 Ident, bias=biasB[:, t_:t_ + 1], scale=1.0)

        def v_proj_group(lt):
            ps = stagePS.tile([128, 512], f32, tag="ps")
            for k in range(4):
                vh = vT_halves[k // 2]
                c0 = (k % 2) * 1024 + lt * 128
                nc.tensor.matmul(
                    ps[:, 0:256],
                    vh[:, c0: c0 + 128],
                    wv_sb[:, k * 256:(k + 1) * 256],
                    start=(k == 0), stop=(k == 3),
                )
            vt = persist.tile([128, 260], bf16, tag=f"vsb{lt}", name=f"vsb{lt}")
            vsrc = ps[:, 0:256].rearrange("p (h d) -> p h d", d=64)
            vdst = vt[:].rearrange("p (h e) -> p h e", e=65)[:, :, 0:64]
            nc.vector.tensor_copy(vdst, vsrc)
            nc.vector.memset(vt[:].rearrange("p (h e) -> p h e", e=65)[:, :, 64:65], 1.0)
            v_sb.append(vt)

        for qi in range(4):
            produce_tile(0, 1, qi)
        q_proj_group(0, 0)
        produce_tile(0, 1, 4)
        q_proj_group(0, 1)
        produce_tile(0, 1, 5)
        q_proj_group(1, 0)
        produce_tile(0, 1, 6)
        q_proj_group(1, 1)
        produce_tile(0, 1, 7)

        # B-produce(0) interleaved with v-proj
        copy_rot[0] = [dve_copy, act_copy]
        for qi in range(8):
            produce_tile(0, 0, qi)
            v_proj_group(qi)
        io_ctx.close()

        produce_fini(0)
        # steady state: DVE takes the 512+128 chunks, ACT one 512 chunk
        copy_rot[0] = [dve_copy, act_copy, dve_copy]

        # ---- pair loop: pair p emits pair p+1's produce tiles ----
        for p in range(4):
            bdall, eshall = pair_state[p][3], pair_state[p][4]
            outT = psO.tile([65, 1024], f32, tag="o", name=f"outT{p}")
            prodlist = []
            if p < 3:
                produce_init(p + 1)
                prodlist = ([(p + 1, 1, qi) for qi in range(8)] +
                            [(p + 1, 0, qi) for qi in range(8)])
            STs = {}
            pi = 0
            for ki in range(8):
                for h in range(2):
                    STs[(ki, h)] = ki_half_head(p, ki, h, bdall, eshall)
                    if ki > 0:
                        ki_half_tail(p, ki - 1, h, STs.pop((ki - 1, h)), outT)
                    if pi < len(prodlist):
                        tp, s, qi = prodlist[pi]; pi += 1
                        produce_tile(tp, s, qi)
                if p < 3 and ki == 7:
                    produce_fini(p + 1)
            for h in range(2):
                ki_half_tail(p, 7, h, STs.pop((7, h)), outT)
            finalize(p, outT)


_CACHE = {}


def _build_nc():
    if "nc" in _CACHE:
        return _CACHE["nc"]
    nc = bacc.Bacc("TRN2", target_bir_lowering=False, debug=False,
                   enable_asserts=False, num_devices=8)
    f32 = mybir.dt.float32
    f16 = mybir.dt.float16
    ins = {}

    def di(name, shape, dt):
        ins[name] = nc.dram_tensor(name, shape, dt, kind="ExternalInput").ap()

    di("qT", [128, 4096], f16)
    di("vT", [128, 4096], f16)
    di("kT", [128, 2048], f16)
    di("Wqv", [128, 2048], f16)
    di("relTa", [128, 2049], f16)
    di("smalls", [128, 270], f32)
    di("ident_f16", [128, 128], f16)
    di("ident_f32", [128, 128], f32)
    outs = {"out": nc.dram_tensor("out", [4, 65, 1024], f32,
                                  kind="ExternalOutput").ap()}

    with tile.TileContext(nc) as tc:
        build_body(tc, ins, outs)
    nc.compile()
    _CACHE["nc"] = nc
    return nc


def make_in_maps(query, key, value, w_q_w, w_q_b, w_v_w, w_v_b, w_r_w, w_r_b,
                 r_r_bias, r_w_bias, seq_len):
    emb = _get_embedding(L, H)
    rel = (emb @ w_r_w.astype(np.float64) + w_r_b.astype(np.float64))  # [2L, 64]
    relTa = np.zeros((128, 2049), dtype=F16)
    relTa[0:64, 0:2048] = rel.T.astype(F16)
    relTa[0:64, 2048] = w_r_b.astype(F16)
    relTa[64:128, :] = relTa[0:64, :]

    ident_f16 = np.eye(128, dtype=F16)
    ident_f32 = np.eye(128, dtype=np.float32)
    seq_len = int(seq_len)
    in_maps = []
    for c in range(8):
        b, hg = c // 2, c % 2
        hs = 256 * hg
        heads = slice(4 * hg, 4 * hg + 4)
        mb = np.where((np.arange(1024) < seq_len), 0.0, NEG).astype(np.float32)
        qT = np.ascontiguousarray(query[b].T).astype(F16)       # [512, 1024]
        vT = np.ascontiguousarray(value[b].T).astype(F16)
        kT = np.ascontiguousarray(key[b][:, hs:hs + 256].T).astype(F16)
        Wq = np.ascontiguousarray(w_q_w[:, hs:hs + 256]).astype(F16)  # [512,256]
        Wv = np.ascontiguousarray(w_v_w[:, hs:hs + 256]).astype(F16)
        # pack: qT -> [128, 4096] (4 chunks of 128 rows side by side)
        qTp = qT.reshape(4, 128, 1024).transpose(1, 0, 2).reshape(128, 4096)
        vTp = vT.reshape(4, 128, 1024).transpose(1, 0, 2).reshape(128, 4096)
        kTp = kT.reshape(2, 128, 1024).transpose(1, 0, 2).reshape(128, 2048)
        Wqp = Wq.reshape(4, 128, 256).transpose(1, 0, 2).reshape(128, 1024)
        Wvp = Wv.reshape(4, 128, 256).transpose(1, 0, 2).reshape(128, 1024)
        smalls = np.zeros((128, 270), dtype=np.float32)
        smalls[:, 0:2] = w_q_b[hs:hs + 256].reshape(2, 128).T
        smalls[:, 2:4] = r_r_bias[heads].reshape(2, 128).T
        smalls[:, 4:6] = r_w_bias[heads].reshape(2, 128).T
        smalls[:, 6:14] = mb.reshape(8, 128).T
        smalls[:, 14:270] = np.tile(w_v_b[hs:hs + 256][None, :], (128, 1))
        in_maps.append({
            "qT": qTp,
            "vT": vTp,
            "kT": kTp,
            "Wqv": np.concatenate([Wqp, Wvp], axis=1),
            "relTa": relTa,
            "smalls": smalls,
            "ident_f16": ident_f16,
            "ident_f32": ident_f32,
        })
    return in_maps


def kernel(query, key, value, w_q_w, w_q_b, w_v_w, w_v_b, w_r_w, w_r_b,
           r_r_bias, r_w_bias, seq_len, _trace=False):
    query = np.asarray(query); key = np.asarray(key); value = np.asarray(value)
    w_q_w = np.asarray(w_q_w); w_q_b = np.asarray(w_q_b)
    w_v_w = np.asarray(w_v_w); w_v_b = np.asarray(w_v_b)
    w_r_w = np.asarray(w_r_w); w_r_b = np.asarray(w_r_b)
    r_r_bias = np.asarray(r_r_bias); r_w_bias = np.asarray(r_w_bias)

    nc = _build_nc()
    in_maps = make_in_maps(query, key, value, w_q_w, w_q_b, w_v_w, w_v_b,
                           w_r_w, w_r_b, r_r_bias, r_w_bias, seq_len)
    res = run_bass_kernel_spmd(nc, in_maps, core_ids=list(range(8)), trace=_trace)
    out = np.zeros((B, L, H), dtype=np.float32)
    for c in range(8):
        b, hg = c // 2, c % 2
        o = res.results[c]["out"]  # [4, 65, 1024]: rows 0-63 attn@vWv^T, 64 denom
        for j in range(4):
            h0 = 256 * hg + 64 * j
            out[b][:, h0:h0 + 64] = (o[j, 0:64, :] / o[j, 64:65, :]).T \
                + w_v_b[h0:h0 + 64][None, :]
    if _trace:
        return out, res
    return out


# revision 5
# speedup vs baseline: 1.8953x; 1.0196x over previous
"""Trainium2 Bass kernel for AdaptSelfAttention (Transformer-XL style relative
position attention) — v2.

Shapes (hardcoded): B=4, L=1024, H=512, NH=8, HD=64.
Sharding: 32 (batch, head) pairs -> 8 cores; core c handles batch c//2 and the
4-head group c%2 (hidden slice of 256 columns).

Math per (b, n):
  q = query @ Wq + bq   (per-head slice)          [L, 64]
  v = value @ Wv + bv                              [L, 64]
  k = key slice                                    [L, 64]
  rel = emb @ Wr + br                              [2L, 64]  (emb = sinusoid const)
  S[q_,k_] = (q+rrb).k  +  (q+rwb).rel[L+k_-q_]  +  k.rel[L+q_-k_]  + c2[k_]
       (c2[k_] = k.br ; the q-side br term is constant per row -> softmax-
        invariant, dropped)
  out = softmax_k(S with key-mask) @ v

v2 design:
  - B_/E_ scratch lives in SBUF (fp16); the shift is ONE SBUF->SBUF skew DMA
    per scratch (diagonal source AP) instead of a DRAM write + skew re-read.
  - S^T accumulates fully on PE: AC matmuls + BD transpose-matmuls (identity
    rhs) + E_sh injection (identity lhsT). No vector adds in the chain; the
    tail is exp (ACT) + AV (PE) only, one ki behind the head.
  - PSUM->SBUF produce copies are rotated over DVE/Pool/ACT.
  - produce(p+1) tiles interleave into kis 0-4 of pair p; skew(p+1) issues at
    ki=4 so its latency hides behind kis 5-7.
  - fp16 everywhere on the logit path (q/k/rel/scratch) for precision
    headroom; P stays bf16 (exp range), v stays bf16.
"""

import math
import sys

import numpy as np

sys.path.insert(0, "/opt/trn_rl_repo")

import concourse.bass as bass
import concourse.tile as tile
from concourse import bacc, mybir
from concourse.bass_utils import run_bass_kernel_spmd

import ml_dtypes

BF16 = ml_dtypes.bfloat16
F16 = np.float16

B, L, H, NH, HD = 4, 1024, 512, 8, 64
PITCH = 1152  # stored l-window width per row of the B_/E_ scratch
NEG = -1e30


def _get_embedding(max_len, dim):
    half = dim // 2
    freq = np.exp(np.arange(half, dtype=np.float64) * (-math.log(10000.0) / (half - 1)))
    pos = np.arange(-max_len, max_len, dtype=np.float64)
    ang = pos[:, None] * freq[None, :]
    return np.concatenate([np.sin(ang), np.cos(ang)], axis=1)


def build_body(tc, ins, outs):
    """Emit the per-core kernel. ins/outs: dicts of bass.AP over DRAM."""
    nc = tc.nc
    f32 = mybir.dt.float32
    bf16 = mybir.dt.bfloat16
    f16 = mybir.dt.float16
    Ident = mybir.ActivationFunctionType.Identity
    Exp = mybir.ActivationFunctionType.Exp

    from contextlib import ExitStack

    ctx = ExitStack()
    with ctx:
        # ---- pools ----
        persist = ctx.enter_context(tc.tile_pool(name="persist", bufs=1))
        sml = ctx.enter_context(tc.tile_pool(name="sml", bufs=2))
        # PSUM banks: stagePS 3x[128,512]=3; psS 3x[128,512]=3; psO 2 -> 8
        stagePS = ctx.enter_context(tc.tile_pool(name="stagePS", bufs=4, space="PSUM"))
        psS = ctx.enter_context(tc.tile_pool(name="psS", bufs=2, space="PSUM"))
        psO = ctx.enter_context(tc.tile_pool(name="psO", bufs=1, space="PSUM"))

        # ---- stage constant/weight inputs into SBUF (one DMA each) ----
        # relT/kT first: they gate the E-produce(0) prologue
        relT = persist.tile([128, 2056], f16, tag="relT")
        nc.scalar.dma_start(relT[:, 0:2049], ins["relTa"])

        kT_t = persist.tile([128, 2048], f16, tag="kT")
        nc.scalar.dma_start(kT_t[:], ins["kT"])
        kT_sb = [kT_t[:, 0:1024], kT_t[:, 1024:2048]]

        # Wq|Wv packed [128, 2048] (wq 4x256, wv 4x256)
        w_sb = persist.tile([128, 2048], f16, tag="w")
        nc.scalar.dma_start(w_sb[:], ins["Wqv"])
        wq_sb = w_sb[:, 0:1024]
        wv_sb = w_sb[:, 1024:2048]

        ident = persist.tile([128, 128], f16, tag="ident")
        nc.scalar.dma_start(ident[:], ins["ident_f16"])

        # smalls packed f32 [128, 270]: bq2|brr2|brw2|maskbias(8)|bv(256)
        smalls = sml.tile([128, 270], f32, tag="smalls")
        nc.scalar.dma_start(smalls[:], ins["smalls"])
        bq_sb = smalls[:, 0:2]
        brr_sb = smalls[:, 2:4]
        brw_sb = smalls[:, 4:6]
        mb_sb = smalls[:, 6:14]
        bv_sb = smalls[:, 14:270]

        biasA = sml.tile([128, 2], f32, tag="biasA")  # bq + r_r_bias
        nc.vector.tensor_add(biasA[:], bq_sb, brr_sb)
        biasB = sml.tile([128, 2], f32, tag="biasB")  # bq + r_w_bias
        nc.vector.tensor_add(biasB[:], bq_sb, brw_sb)

        qrrT = [persist.tile([128, 1024], f16, tag=f"qrrT{i}", name=f"qrrT{i}")
                for i in range(2)]
        grwT = [persist.tile([128, 1024], f16, tag=f"grwT{i}", name=f"grwT{i}")
                for i in range(2)]
        v_sb = []

        scrp = ctx.enter_context(tc.tile_pool(name="scrp", bufs=1))   # SBUF scratch
        bdp = ctx.enter_context(tc.tile_pool(name="bdp", bufs=2))     # skewed bd
        ep = ctx.enter_context(tc.tile_pool(name="ep", bufs=2))       # skewed esh
        pp = ctx.enter_context(tc.tile_pool(name="pp", bufs=4))       # exp outputs

        # produce-copy engines. GPSIMD/Pool cannot read PSUM on TRN2, so
        # the PSUM->SBUF copies rotate over DVE and ACT only.
        dve_copy = lambda o_, i_: nc.vector.tensor_copy(o_, i_)
        act_copy = lambda o_, i_: nc.scalar.copy(o_, i_)
        copy_rot = [[dve_copy, act_copy]]
        copy_ctr = [0]

        # ---- per-(b,head) pair machinery ----
        pair_state = {}

        def produce_init(p):
            # one scratch tile per (src_sel, qi) chunk so each chunk's skew
            # DMA read only depends on that chunk's copies
            scr = {(s, qi): scrp.tile([128, PITCH], f16, tag=f"scr{s}_{qi}",
                                      name=f"scr{p}_{s}_{qi}")
                   for s in range(2) for qi in range(8)}
            c2 = sml.tile([128, 8], f32, tag="c2", name=f"c2_{p}")
            ebias = sml.tile([128, 8], f32, tag="ebias", name=f"ebias{p}")
            bdall = bdp.tile([128, 8192], f16, tag="bd", name=f"bdall{p}")
            eshall = ep.tile([128, 8192], f16, tag="esh", name=f"esh{p}")
            pair_state[p] = (scr, c2, ebias, bdall, eshall)
            return bdall, eshall

        def produce_tile(p, src_sel, qi):
            """One B_ (src_sel=0) or E_ (1) tile of pair p -> SBUF scratch."""
            t_ = p // 2
            o = (p % 2) * 64
            scrd, c2, ebias, bdall, eshall = pair_state[p]
            scr = scrd[(src_sel, qi)]
            W0 = 897 - 128 * qi
            if src_sel == 0:
                lhs = grwT[t_][o:o + 64, qi * 128:(qi + 1) * 128]
            else:
                lhs = kT_sb[t_][o:o + 64, qi * 128:(qi + 1) * 128]
            for ci, (c0, cw) in enumerate(((0, 512), (512, 512), (1024, 128))):
                ps = stagePS.tile([128, 512], f32, tag="ps")
                nc.tensor.matmul(ps[:, 0:cw], lhs,
                                 relT[o:o + 64, W0 + c0:W0 + c0 + cw],
                                 start=True, stop=True)
                if src_sel == 1 and ci == 2:
                    nc.tensor.matmul(ps[:, 128:129], lhs,
                                     relT[o:o + 64, 2048:2049],
                                     start=True, stop=True)
                    nc.vector.tensor_copy(c2[:, qi:qi + 1], ps[:, 128:129])
                rot = copy_rot[0]
                eng = rot[copy_ctr[0] % len(rot)]
                copy_ctr[0] += 1
                eng(scr[:][:, c0:c0 + cw], ps[:, 0:cw])
            # this chunk of the scratch is complete: issue its skew DMA now
            # skew: dst[p', i] = scr[p', 127 - p' + i]
            dst = bdall if src_sel == 0 else eshall
            scr_ap = scr[:]
            src_ap = bass.AP(scr_ap.tensor, scr_ap.offset + 127,
                             [[PITCH - 1, 128], [1, 1024]])
            nc.sync.dma_start(dst[:, qi * 1024:(qi + 1) * 1024], src_ap)

        def produce_fini(p):
            c2, ebias = pair_state[p][1], pair_state[p][2]
            nc.vector.tensor_add(ebias[:], c2[:], mb_sb)

        def ki_half_head(p, ki, h, bdall, eshall):
            """One 512-col half of ST = AC^T + BD_sh^T + E_sh^T (all PE)."""
            t_ = p // 2
            o = (p % 2) * 64
            kTs = kT_sb[t_]
            STh = psS.tile([128, 512], f32, tag="sh", name=f"ST{p}_{ki}_{h}")
            nc.tensor.matmul(
                STh[:],
                kTs[o:o + 64, ki * 128:(ki + 1) * 128],
                qrrT[t_][o:o + 64, h * 512:(h + 1) * 512],
                start=True, stop=False,
            )
            for j in range(4):
                qi = 4 * h + j
                nc.tensor.matmul(
                    STh[:, j * 128:(j + 1) * 128],
                    bdall[:, qi * 1024 + ki * 128: qi * 1024 + (ki + 1) * 128],
                    ident[:],
                    start=False, stop=False,
                )
            nc.tensor.matmul(
                STh[:],
                ident[:],
                eshall[:, ki * 1024 + h * 512: ki * 1024 + (h + 1) * 512],
                start=False, stop=True,
            )
            return STh

        def ki_half_tail(p, ki, h, STh, outT):
            ebias = pair_state[p][2]
            Ph = pp.tile([128, 512], bf16, tag="ph", name=f"P{p}_{ki}_{h}")
            nc.scalar.activation(Ph[:], STh[:], Exp,
                                 bias=ebias[:, ki:ki + 1], scale=1.0)
            nc.tensor.matmul(
                outT[0:65, h * 512:(h + 1) * 512],
                v_sb[ki][:, p * 65:p * 65 + 65],
                Ph[:],
                start=(ki == 0), stop=(ki == 7),
            )

        def finalize(p, outT):
            # ship the raw [65, 1024] accumulator ([attn@vWv ; denom] rows);
            # the host divides, transposes and adds bv
            oT = sml.tile([65, 1024], f32, tag="oT", name=f"oT{p}")
            nc.vector.tensor_copy(oT[:], outT[:])
            nc.sync.dma_start(outs["out"][p], oT[:])

        # ================= emission schedule =================
        # E-produce(0) first (needs only kT/relT); q-proj groups interleave
        # once qT arrives; B(0) and v-proj interleave after.
        produce_init(0)

        io_ctx = ExitStack()
        io = io_ctx.enter_context(tc.tile_pool(name="io", bufs=1))
        qT_sb = io.tile([128, 4096], f16, tag="qT")
        nc.scalar.dma_start(qT_sb[:, 0:2048], ins["qT"][:, 0:2048])
        nc.scalar.dma_start(qT_sb[:, 2048:4096], ins["qT"][:, 2048:4096])
        qT_halves = [qT_sb[:, 0:2048], qT_sb[:, 2048:4096]]
        vT_sb = io.tile([128, 4096], f16, tag="vT")
        nc.scalar.dma_start(vT_sb[:, 0:2048], ins["vT"][:, 0:2048])
        nc.scalar.dma_start(vT_sb[:, 2048:4096], ins["vT"][:, 2048:4096])
        vT_halves = [vT_sb[:, 0:2048], vT_sb[:, 2048:4096]]

        def q_proj_group(t_, nh):
            ps = stagePS.tile([128, 512], f32, tag="ps")
            for k in range(4):
                qh = qT_halves[k // 2]
                c0 = (k % 2) * 1024 + nh * 512
                nc.tensor.matmul(
                    ps[:],
                    wq_sb[:, k * 256 + t_ * 128: k * 256 + (t_ + 1) * 128],
                    qh[:, c0: c0 + 512],
                    start=(k == 0), stop=(k == 3),
                )
            nc.scalar.activation(qrrT[t_][:, nh * 512:(nh + 1) * 512], ps[:],
                                 Ident, bias=biasA[:, t_:t_ + 1], scale=1.0)
            nc.scalar.activation(grwT[t_][:, nh * 512:(nh + 1) * 512], ps[:],
                                 Ident, bias=biasB[:, t_:t_ + 1], scale=1.0)

        def v_proj_group(lt):
            ps = stagePS.tile([128, 512], f32, tag="ps")
            for k in range(4):
                vh = vT_halves[k // 2]
                c0 = (k % 2) * 1024 + lt * 128
                nc.tensor.matmul(
                    ps[:, 0:256],
                    vh[:, c0: c0 + 128],
                    wv_sb[:, k * 256:(k + 1) * 256],
                    start=(k == 0), stop=(k == 3),
                )
            vt = persist.tile([128, 260], bf16, tag=f"vsb{lt}", name=f"vsb{lt}")
            vsrc = ps[:, 0:256].rearrange("p (h d) -> p h d", d=64)
            vdst = vt[:].rearrange("p (h e) -> p h e", e=65)[:, :, 0:64]
            nc.vector.tensor_copy(vdst, vsrc)
            nc.vector.memset(vt[:].rearrange("p (h e) -> p h e", e=65)[:, :, 64:65], 1.0)
            v_sb.append(vt)

        for qi in range(4):
            produce_tile(0, 1, qi)
        q_proj_group(0, 0)
        produce_tile(0, 1, 4)
        q_proj_group(0, 1)
        produce_tile(0, 1, 5)
        q_proj_group(1, 0)
        produce_tile(0, 1, 6)
        q_proj_group(1, 1)
        produce_tile(0, 1, 7)

        # B-produce(0) interleaved with v-proj
        copy_rot[0] = [dve_copy, act_copy]
        for qi in range(8):
            produce_tile(0, 0, qi)
            v_proj_group(qi)
        io_ctx.close()

        produce_fini(0)
        # steady state: DVE takes the 512+128 chunks, ACT one 512 chunk
        copy_rot[0] = [dve_copy, act_copy, dve_copy]

        # ---- pair loop: pair p emits pair p+1's produce tiles ----
        for p in range(4):
            bdall, eshall = pair_state[p][3], pair_state[p][4]
            outT = psO.tile([65, 1024], f32, tag="o", name=f"outT{p}")
            prodlist = []
            if p < 3:
                produce_init(p + 1)
                prodlist = ([(p + 1, 1, qi) for qi in range(8)] +
                            [(p + 1, 0, qi) for qi in range(8)])
            STs = {}
            pi = 0
            for ki in range(8):
                for h in range(2):
                    STs[(ki, h)] = ki_half_head(p, ki, h, bdall, eshall)
                    if ki > 0:
                        ki_half_tail(p, ki - 1, h, STs.pop((ki - 1, h)), outT)
                    if pi < len(prodlist):
                        tp, s, qi = prodlist[pi]; pi += 1
                        produce_tile(tp, s, qi)
                if p < 3 and ki == 7:
                    produce_fini(p + 1)
            for h in range(2):
                ki_half_tail(p, 7, h, STs.pop((7, h)), outT)
            finalize(p, outT)


_CACHE = {}


def _build_nc():
    if "nc" in _CACHE:
        return _CACHE["nc"]
    nc = bacc.Bacc("TRN2", target_bir_lowering=False, debug=False,
                   enable_asserts=False, num_devices=8)
    f32 = mybir.dt.float32
    f16 = mybir.dt.float16
    ins = {}

    def di(name, shape, dt):
        ins[name] = nc.dram_tensor(name, shape, dt, kind="ExternalInput").ap()

    di("qT", [128, 4096], f16)
    di("vT", [128, 4096], f16)
    di("kT", [128, 2048], f16)
    di("Wqv", [128, 2048], f16)
    di("relTa", [128, 2049], f16)
    di("smalls", [128, 270], f32)
    di("ident_f16", [128, 128], f16)
    outs = {"out": nc.dram_tensor("out", [4, 65, 1024], f32,
                                  kind="ExternalOutput").ap()}

    with tile.TileContext(nc) as tc:
        build_body(tc, ins, outs)
    nc.compile()
    _CACHE["nc"] = nc
    return nc


def make_in_maps(query, key, value, w_q_w, w_q_b, w_v_w, w_v_b, w_r_w, w_r_b,
                 r_r_bias, r_w_bias, seq_len):
    emb = _get_embedding(L, H)
    rel = (emb @ w_r_w.astype(np.float64) + w_r_b.astype(np.float64))  # [2L, 64]
    relTa = np.zeros((128, 2049), dtype=F16)
    relTa[0:64, 0:2048] = rel.T.astype(F16)
    relTa[0:64, 2048] = w_r_b.astype(F16)
    relTa[64:128, :] = relTa[0:64, :]

    ident_f16 = np.eye(128, dtype=F16)
    seq_len = int(seq_len)
    in_maps = []
    for c in range(8):
        b, hg = c // 2, c % 2
        hs = 256 * hg
        heads = slice(4 * hg, 4 * hg + 4)
        mb = np.where((np.arange(1024) < seq_len), 0.0, NEG).astype(np.float32)
        qT = np.ascontiguousarray(query[b].T).astype(F16)       # [512, 1024]
        vT = np.ascontiguousarray(value[b].T).astype(F16)
        kT = np.ascontiguousarray(key[b][:, hs:hs + 256].T).astype(F16)
        Wq = np.ascontiguousarray(w_q_w[:, hs:hs + 256]).astype(F16)  # [512,256]
        Wv = np.ascontiguousarray(w_v_w[:, hs:hs + 256]).astype(F16)
        # pack: qT -> [128, 4096] (4 chunks of 128 rows side by side)
        qTp = qT.reshape(4, 128, 1024).transpose(1, 0, 2).reshape(128, 4096)
        vTp = vT.reshape(4, 128, 1024).transpose(1, 0, 2).reshape(128, 4096)
        kTp = kT.reshape(2, 128, 1024).transpose(1, 0, 2).reshape(128, 2048)
        Wqp = Wq.reshape(4, 128, 256).transpose(1, 0, 2).reshape(128, 1024)
        Wvp = Wv.reshape(4, 128, 256).transpose(1, 0, 2).reshape(128, 1024)
        smalls = np.zeros((128, 270), dtype=np.float32)
        smalls[:, 0:2] = w_q_b[hs:hs + 256].reshape(2, 128).T
        smalls[:, 2:4] = r_r_bias[heads].reshape(2, 128).T
        smalls[:, 4:6] = r_w_bias[heads].reshape(2, 128).T
        smalls[:, 6:14] = mb.reshape(8, 128).T
        smalls[:, 14:270] = np.tile(w_v_b[hs:hs + 256][None, :], (128, 1))
        in_maps.append({
            "qT": qTp,
            "vT": vTp,
            "kT": kTp,
            "Wqv": np.concatenate([Wqp, Wvp], axis=1),
            "relTa": relTa,
            "smalls": smalls,
            "ident_f16": ident_f16,
        })
    return in_maps


def kernel(query, key, value, w_q_w, w_q_b, w_v_w, w_v_b, w_r_w, w_r_b,
           r_r_bias, r_w_bias, seq_len, _trace=False):
    query = np.asarray(query); key = np.asarray(key); value = np.asarray(value)
    w_q_w = np.asarray(w_q_w); w_q_b = np.asarray(w_q_b)
    w_v_w = np.asarray(w_v_w); w_v_b = np.asarray(w_v_b)
    w_r_w = np.asarray(w_r_w); w_r_b = np.asarray(w_r_b)
    r_r_bias = np.asarray(r_r_bias); r_w_bias = np.asarray(r_w_bias)

    nc = _build_nc()
    in_maps = make_in_maps(query, key, value, w_q_w, w_q_b, w_v_w, w_v_b,
                           w_r_w, w_r_b, r_r_bias, r_w_bias, seq_len)
    res = run_bass_kernel_spmd(nc, in_maps, core_ids=list(range(8)), trace=_trace)
    out = np.zeros((B, L, H), dtype=np.float32)
    for c in range(8):
        b, hg = c // 2, c % 2
        o = res.results[c]["out"]  # [4, 65, 1024]: rows 0-63 attn@vWv^T, 64 denom
        for j in range(4):
            h0 = 256 * hg + 64 * j
            out[b][:, h0:h0 + 64] = (o[j, 0:64, :] / o[j, 64:65, :]).T \
                + w_v_b[h0:h0 + 64][None, :]
    if _trace:
        return out, res
    return out
